# revision 5
# baseline (speedup 1.0000x reference)
"""JK-GAMLP forward on 8 Trainium2 NeuronCores (Bass/Tile).

Strategy: shard nodes across 8 cores; each core runs the whole per-node
network on node tiles of 512 (blocks of 128 on partitions).  Activations
live feature-major ("transposed", [feat, node]) so every Dense contraction
is a PE matmul; the attention softmax and hop aggregation run node-major
where the per-node weights are per-partition scalars.  Matmuls use the
fp32r PE mode (1 cycle/row); transposes use exact fp32 PE transposes.
"""
import numpy as np

import concourse.bacc as bacc
import concourse.mybir as mybir
import concourse.tile as tile
from concourse.bass_utils import run_bass_kernel_spmd

AF = mybir.ActivationFunctionType
ALU = mybir.AluOpType
F32 = mybir.dt.float32
F32R = mybir.dt.float32r

HOPS, F, HID, NCLS = 8, 128, 256, 64
N = 100000
NCORES = 8
NPC = 12544                       # nodes per core (padded: 8*12544 = 100352)
TILES = [(i * 512, 512) for i in range(24)] + [(12288, 256)]

_CACHE = {}


def _build_program():
    nc = bacc.Bacc("TRN2", target_bir_lowering=False, debug=False,
                   num_devices=NCORES)

    feats = nc.dram_tensor("feats", [HOPS, NPC, F], F32, kind="ExternalInput")
    W_jk1 = nc.dram_tensor("W_jk1", [HOPS * F, HID], F32, kind="ExternalInput")
    W_jk2 = nc.dram_tensor("W_jk2", [HID, HID], F32, kind="ExternalInput")
    w_att_ref = nc.dram_tensor("w_att_ref", [HID], F32, kind="ExternalInput")
    w_att_x = nc.dram_tensor("w_att_x", [F], F32, kind="ExternalInput")
    W_o1 = nc.dram_tensor("W_o1", [F, HID], F32, kind="ExternalInput")
    W_o2 = nc.dram_tensor("W_o2", [HID, NCLS], F32, kind="ExternalInput")
    a_jk = nc.dram_tensor("a_jk", [1, 1], F32, kind="ExternalInput")
    a_main = nc.dram_tensor("a_main", [1, 1], F32, kind="ExternalInput")
    a_out = nc.dram_tensor("a_out", [1, 1], F32, kind="ExternalInput")
    out = nc.dram_tensor("out", [NPC, NCLS], F32, kind="ExternalOutput")

    ident = nc.inline_tensor(np.eye(128, dtype=np.float32), name="ident")

    with tile.TileContext(nc) as tc:
        with tc.tile_pool(name="const", bufs=1) as cpool, \
             tc.tile_pool(name="x", bufs=20) as xpool, \
             tc.tile_pool(name="xt", bufs=16) as xtpool, \
             tc.tile_pool(name="act", bufs=4) as actpool, \
             tc.tile_pool(name="tmp", bufs=6) as tmppool, \
             tc.tile_pool(name="sm", bufs=4) as smpool, \
             tc.tile_pool(name="aggp", bufs=24) as aggpool, \
             tc.tile_pool(name="outp", bufs=2) as outpool, \
             tc.tile_pool(name="xt_ps", bufs=2, space="PSUM") as xtps, \
             tc.tile_pool(name="mm_ps", bufs=2, space="PSUM") as mmps, \
             tc.tile_pool(name="s_ps", bufs=1, space="PSUM") as sps, \
             tc.tile_pool(name="e_ps", bufs=1, space="PSUM") as eps, \
             tc.tile_pool(name="at_ps", bufs=1, space="PSUM") as atps, \
             tc.tile_pool(name="o_ps", bufs=1, space="PSUM") as ops_:

            # ---------------- setup: weights + constants ----------------
            id_sb = cpool.tile([128, 128], F32)
            nc.sync.dma_start(id_sb[:], ident[:])

            w1f = cpool.tile([128, HOPS, 2, 128], F32)
            nc.sync.dma_start(
                w1f[:], W_jk1.ap().rearrange("(h p) (m c) -> p h m c",
                                             p=128, m=2))
            w1r = cpool.tile([128, HOPS, 2, 128], F32R)
            nc.vector.tensor_copy(w1r[:], w1f[:])

            w2f = cpool.tile([128, 2, 2, 128], F32)
            nc.sync.dma_start(
                w2f[:], W_jk2.ap().rearrange("(k p) (m c) -> p k m c",
                                             p=128, m=2))
            w2r = cpool.tile([128, 2, 2, 128], F32R)
            nc.vector.tensor_copy(w2r[:], w2f[:])

            wo1f = cpool.tile([128, 2, 128], F32)
            nc.sync.dma_start(
                wo1f[:], W_o1.ap().rearrange("p (m c) -> p m c", m=2))
            wo1r = cpool.tile([128, 2, 128], F32R)
            nc.vector.tensor_copy(wo1r[:], wo1f[:])

            wo2f = cpool.tile([128, 2, NCLS], F32)
            nc.sync.dma_start(
                wo2f[:], W_o2.ap().rearrange("(k p) c -> p k c", p=128))
            wo2r = cpool.tile([128, 2, NCLS], F32R)
            nc.vector.tensor_copy(wo2r[:], wo2f[:])

            wreff = cpool.tile([128, 2], F32)
            nc.sync.dma_start(wreff[:],
                              w_att_ref.ap().rearrange("(k p) -> p k", p=128))
            wref8 = cpool.tile([128, 2, 8], F32R)
            for j in range(8):
                nc.vector.tensor_copy(wref8[:, :, j], wreff[:])

            watxf = cpool.tile([128, 1], F32)
            nc.sync.dma_start(watxf[:],
                              w_att_x.ap().rearrange("(p o) -> p o", o=1))
            watx8f = cpool.tile([128, HOPS, 8], F32)
            nc.vector.memset(watx8f[:], 0.0)
            for h in range(HOPS):
                nc.vector.tensor_copy(watx8f[:, h, h:h + 1], watxf[:])
            watx8 = cpool.tile([128, HOPS, 8], F32R)
            nc.vector.tensor_copy(watx8[:], watx8f[:])

            # replicate the three PReLU alphas to [128, 3] via K=1 matmul
            al_f = cpool.tile([1, 3], F32)
            nc.sync.dma_start(al_f[0:1, 0:1], a_jk[:])
            nc.sync.dma_start(al_f[0:1, 1:2], a_main[:])
            nc.sync.dma_start(al_f[0:1, 2:3], a_out[:])
            ones_sb = cpool.tile([1, 128], F32)
            nc.vector.memset(ones_sb[:], 1.0)
            al_ps = mmps.tile([128, 3], F32, tag="mm")
            nc.tensor.matmul(al_ps[:], ones_sb[:], al_f[:],
                             start=True, stop=True)
            alpha = cpool.tile([128, 3], F32)
            nc.scalar.activation(alpha[:], al_ps[:], AF.Copy)

            def prelu_from_psum(dst, ps_ap, a_col):
                """dst = max(ps, a*ps); ACT does the scaled copy, DVE the max."""
                t = tmppool.tile(list(dst.shape), F32, tag="axtmp")
                nc.scalar.activation(t[:], ps_ap, AF.Copy, scale=a_col)
                nc.vector.scalar_tensor_tensor(
                    dst[:], ps_ap, 1.0, t[:], op0=ALU.mult, op1=ALU.max)

            # ---------------- main loop over node tiles ----------------
            for n0, TT in TILES:
                B = TT // 128

                # load + transpose each hop
                x_sb = []
                xt_sb = []
                for h in range(HOPS):
                    x_h = xpool.tile([128, B, 128], F32, tag="x")
                    nc.sync.dma_start(
                        x_h[:],
                        feats.ap()[h, n0:n0 + TT, :].rearrange(
                            "(b p) f -> p b f", p=128))
                    x_sb.append(x_h)

                    ps = xtps.tile([128, TT], F32, tag="xtps")
                    for b in range(B):
                        nc.tensor.transpose(ps[:, b * 128:(b + 1) * 128],
                                            x_h[:, b, :], id_sb[:])
                    xt_h = xtpool.tile([128, TT], F32R, tag="xt")
                    nc.scalar.activation(xt_h[:], ps[:], AF.Copy)
                    xt_sb.append(xt_h)

                # JK layer 1: h1T[m] = prelu(sum_h W1[h,m].T @ xT_h, a_jk)
                h1_sb = []
                for m in range(2):
                    ps = mmps.tile([128, TT], F32, tag="mm")
                    for h in range(HOPS):
                        nc.tensor.matmul(ps[:], w1r[:, h, m, :], xt_sb[h][:],
                                         start=(h == 0), stop=(h == HOPS - 1))
                    h1 = actpool.tile([128, TT], F32R, tag="h1")
                    prelu_from_psum(h1, ps[:], alpha[:, 0:1])
                    h1_sb.append(h1)

                # JK layer 2: jkT[m] = prelu(sum_k W2[k,m].T @ h1T_k, a_main)
                jk_sb = []
                for m in range(2):
                    ps = mmps.tile([128, TT], F32, tag="mm")
                    for k in range(2):
                        nc.tensor.matmul(ps[:], w2r[:, k, m, :], h1_sb[k][:],
                                         start=(k == 0), stop=(k == 1))
                    jk = actpool.tile([128, TT], F32R, tag="jk")
                    prelu_from_psum(jk, ps[:], alpha[:, 1:2])
                    jk_sb.append(jk)

                # attention scores [8, TT]: every row gets s_ref, row h gets
                # s_x_h via the column-delta lhsT
                s_ps = sps.tile([8, TT], F32, tag="sps")
                for k in range(2):
                    nc.tensor.matmul(s_ps[:], wref8[:, k, :], jk_sb[k][:],
                                     start=(k == 0), stop=False,
                                     skip_group_check=True)
                for h in range(HOPS):
                    nc.tensor.matmul(s_ps[:], watx8[:, h, :], xt_sb[h][:],
                                     start=False, stop=(h == HOPS - 1),
                                     skip_group_check=True)
                sg = smpool.tile([8, TT], F32, tag="sg")
                nc.scalar.activation(sg[:], s_ps[:], AF.Sigmoid)

                # transpose scores to node-major, softmax over hops
                e_ps = eps.tile([128, B * 8], F32, tag="eps")
                for b in range(B):
                    nc.tensor.transpose(e_ps[:, b * 8:(b + 1) * 8],
                                        sg[:, b * 128:(b + 1) * 128],
                                        id_sb[0:8, 0:8])
                e_sb = smpool.tile([128, B, 8], F32, tag="e")
                esum = smpool.tile([128, B], F32, tag="esum")
                for b in range(B):
                    nc.scalar.activation(e_sb[:, b, :], e_ps[:, b * 8:(b + 1) * 8],
                                         AF.Exp, accum_out=esum[:, b:b + 1])
                r_sb = smpool.tile([128, B], F32, tag="r")
                nc.vector.reciprocal(r_sb[:], esum[:])
                ew = smpool.tile([128, B, 8], F32, tag="ew")
                for b in range(B):
                    nc.vector.tensor_scalar(ew[:, b, :], e_sb[:, b, :],
                                            r_sb[:, b:b + 1], None,
                                            op0=ALU.mult)

                # weighted hop aggregation, node-major: per-hop scaled copies
                # (ACT/DVE alternating), then an add tree on GPSIMD
                agg_blocks = []
                for b in range(B):
                    prods = []
                    for h in range(HOPS):
                        t = aggpool.tile([128, 128], F32, tag="aggp")
                        if h % 2 == 0:
                            nc.scalar.activation(t[:], x_sb[h][:, b, :],
                                                 AF.Copy,
                                                 scale=ew[:, b, h:h + 1])
                        else:
                            nc.vector.tensor_scalar(t[:], x_sb[h][:, b, :],
                                                    ew[:, b, h:h + 1], None,
                                                    op0=ALU.mult)
                        prods.append(t)
                    while len(prods) > 1:
                        nxt_level = []
                        for i in range(0, len(prods), 2):
                            s = aggpool.tile([128, 128], F32, tag="aggp")
                            nc.gpsimd.tensor_tensor(
                                s[:], prods[i][:], prods[i + 1][:], ALU.add)
                            nxt_level.append(s)
                        prods = nxt_level
                    agg_blocks.append(prods[0])

                # transpose agg back to feature-major
                a_ps = atps.tile([128, TT], F32, tag="atps")
                for b in range(B):
                    nc.tensor.transpose(a_ps[:, b * 128:(b + 1) * 128],
                                        agg_blocks[b][:], id_sb[:])
                aggt = actpool.tile([128, TT], F32R, tag="aggt")
                nc.scalar.activation(aggt[:], a_ps[:], AF.Copy)

                # output FFN layer 1: o1T[m] = prelu(Wo1[m].T @ aggT, a_out)
                o1_sb = []
                for m in range(2):
                    ps = mmps.tile([128, TT], F32, tag="mm")
                    nc.tensor.matmul(ps[:], wo1r[:, m, :], aggt[:],
                                     start=True, stop=True)
                    o1 = actpool.tile([128, TT], F32R, tag="o1")
                    prelu_from_psum(o1, ps[:], alpha[:, 2:3])
                    o1_sb.append(o1)

                # output layer 2, node-major out: out[b] = sum_k o1T_k[b].T @ Wo2_k
                o_ps = ops_.tile([128, B * NCLS], F32, tag="ops")
                for b in range(B):
                    for k in range(2):
                        nc.tensor.matmul(
                            o_ps[:, b * NCLS:(b + 1) * NCLS],
                            o1_sb[k][:, b * 128:(b + 1) * 128],
                            wo2r[:, k, :],
                            start=(k == 0), stop=(k == 1),
                            skip_group_check=True)
                out_sb = outpool.tile([128, B, NCLS], F32, tag="out")
                nc.scalar.activation(out_sb[:], o_ps[:], AF.Copy)
                nc.sync.dma_start(
                    out.ap()[n0:n0 + TT, :].rearrange("(b p) c -> p b c",
                                                      p=128),
                    out_sb[:])

    nc.compile()
    return nc


def _get_program():
    if "nc" not in _CACHE:
        _CACHE["nc"] = _build_program()
    return _CACHE["nc"]


def kernel(**inputs):
    nc = _get_program()

    feats = np.asarray(inputs["feats"], dtype=np.float32)
    pad = NCORES * NPC - feats.shape[1]
    feats_p = np.pad(feats, ((0, 0), (0, pad), (0, 0)))

    def scal(name):
        return np.asarray(inputs[name], dtype=np.float32).reshape(1, 1)

    shared = {
        "W_jk1": np.ascontiguousarray(inputs["W_jk1"], dtype=np.float32),
        "W_jk2": np.ascontiguousarray(inputs["W_jk2"], dtype=np.float32),
        "w_att_ref": np.ascontiguousarray(inputs["w_att_ref"], dtype=np.float32),
        "w_att_x": np.ascontiguousarray(inputs["w_att_x"], dtype=np.float32),
        "W_o1": np.ascontiguousarray(inputs["W_o1"], dtype=np.float32),
        "W_o2": np.ascontiguousarray(inputs["W_o2"], dtype=np.float32),
        "a_jk": scal("a_jk"), "a_main": scal("a_main"), "a_out": scal("a_out"),
    }
    in_maps = []
    for c in range(NCORES):
        m = dict(shared)
        m["feats"] = np.ascontiguousarray(feats_p[:, c * NPC:(c + 1) * NPC, :])
        in_maps.append(m)

    res = run_bass_kernel_spmd(nc, in_maps, core_ids=list(range(NCORES)))
    out = np.concatenate([res.results[c]["out"] for c in range(NCORES)],
                         axis=0)[:N]
    return np.ascontiguousarray(out, dtype=np.float32)


# revision 13
# speedup vs baseline: 181.7206x; 181.7206x over previous
"""JK-GAMLP forward on 8 Trainium2 NeuronCores (Bass/Tile).

Strategy: shard nodes across 8 cores; each core runs the whole per-node
network on node tiles of 512 (blocks of 128 on partitions).  Activations
live feature-major ("transposed", [feat, node]) so every Dense contraction
is a PE matmul; the attention softmax and hop aggregation run node-major
where the per-node weights are per-partition scalars.  Matmuls use the
fp32r PE mode (1 cycle/row); transposes use exact fp32 PE transposes.
"""
import numpy as np

import concourse.bacc as bacc
import concourse.mybir as mybir
import concourse.tile as tile
from concourse.bass_utils import run_bass_kernel_spmd

AF = mybir.ActivationFunctionType
ALU = mybir.AluOpType
F32 = mybir.dt.float32
F32R = mybir.dt.float32r

HOPS, F, HID, NCLS = 8, 128, 256, 64
N = 100000
NCORES = 8
NPC = 12544                       # nodes per core (padded: 8*12544 = 100352)
TILES = [(i * 512, 512) for i in range(24)] + [(12288, 256)]

_CACHE = {}

# load feats as f32r so the PE transposes run in f32r mode (1.5 vs 2 cyc/row)
XR_LOAD = True


def _build_program(loop_k=None):
    """loop_k: wrap the whole tile loop in a hardware For_i that repeats it
    loop_k times (identical work, for timing amplification)."""
    nc = bacc.Bacc("TRN2", target_bir_lowering=False, debug=False,
                   num_devices=NCORES)

    feats = nc.dram_tensor("feats", [HOPS, NPC, F],
                           F32R if XR_LOAD else F32, kind="ExternalInput")
    W_jk1 = nc.dram_tensor("W_jk1", [HOPS * F, HID], F32, kind="ExternalInput")
    W_jk2 = nc.dram_tensor("W_jk2", [HID, HID], F32, kind="ExternalInput")
    w_att_ref = nc.dram_tensor("w_att_ref", [HID], F32, kind="ExternalInput")
    w_att_x = nc.dram_tensor("w_att_x", [F], F32, kind="ExternalInput")
    W_o1 = nc.dram_tensor("W_o1", [F, HID], F32, kind="ExternalInput")
    W_o2 = nc.dram_tensor("W_o2", [HID, NCLS], F32, kind="ExternalInput")
    a_jk = nc.dram_tensor("a_jk", [1, 1], F32, kind="ExternalInput")
    a_main = nc.dram_tensor("a_main", [1, 1], F32, kind="ExternalInput")
    a_out = nc.dram_tensor("a_out", [1, 1], F32, kind="ExternalInput")
    out = nc.dram_tensor("out", [NPC, NCLS], F32, kind="ExternalOutput")

    ident = nc.inline_tensor(np.eye(128, dtype=np.float32), name="ident")

    with tile.TileContext(nc) as tc:
        with tc.tile_pool(name="const", bufs=1) as cpool, \
             tc.tile_pool(name="x", bufs=28) as xpool, \
             tc.tile_pool(name="xt", bufs=16) as xtpool, \
             tc.tile_pool(name="act", bufs=4) as actpool, \
             tc.tile_pool(name="tmp", bufs=6) as tmppool, \
             tc.tile_pool(name="sm", bufs=6) as smpool, \
             tc.tile_pool(name="aggp", bufs=24) as aggpool, \
             tc.tile_pool(name="outp", bufs=3) as outpool, \
             tc.tile_pool(name="xt_ps", bufs=2, space="PSUM") as xtps, \
             tc.tile_pool(name="mm_ps", bufs=2, space="PSUM") as mmps, \
             tc.tile_pool(name="mm3_ps", bufs=2, space="PSUM") as mm3ps, \
             tc.tile_pool(name="e_ps", bufs=2, space="PSUM") as eps:

            # ---------------- setup: weights + constants ----------------
            id_sb = cpool.tile([128, 128], F32)
            nc.sync.dma_start(id_sb[:], ident[:])
            XDT = F32R if XR_LOAD else F32

            w1f = cpool.tile([128, HOPS, 2, 128], F32)
            nc.sync.dma_start(
                w1f[:], W_jk1.ap().rearrange("(h p) (m c) -> p h m c",
                                             p=128, m=2))
            w1r = cpool.tile([128, HOPS, 2, 128], F32R)
            nc.vector.tensor_copy(w1r[:], w1f[:])

            w2f = cpool.tile([128, 2, 2, 128], F32)
            nc.sync.dma_start(
                w2f[:], W_jk2.ap().rearrange("(k p) (m c) -> p k m c",
                                             p=128, m=2))
            w2r = cpool.tile([128, 2, 2, 128], F32R)
            nc.vector.tensor_copy(w2r[:], w2f[:])

            wo1f = cpool.tile([128, 2, 128], F32)
            nc.sync.dma_start(
                wo1f[:], W_o1.ap().rearrange("p (m c) -> p m c", m=2))
            wo1r = cpool.tile([128, 2, 128], F32R)
            nc.vector.tensor_copy(wo1r[:], wo1f[:])

            wo2f = cpool.tile([128, 2, NCLS], F32)
            nc.sync.dma_start(
                wo2f[:], W_o2.ap().rearrange("(k p) c -> p k c", p=128))
            wo2r = cpool.tile([128, 2, NCLS], F32R)
            nc.vector.tensor_copy(wo2r[:], wo2f[:])

            wreff = cpool.tile([128, 2], F32)
            nc.sync.dma_start(wreff[:],
                              w_att_ref.ap().rearrange("(k p) -> p k", p=128))
            wref8 = cpool.tile([128, 2, 8], F32R)
            for j in range(8):
                nc.vector.tensor_copy(wref8[:, :, j], wreff[:])

            watxf = cpool.tile([128, 1], F32)
            nc.sync.dma_start(watxf[:],
                              w_att_x.ap().rearrange("(p o) -> p o", o=1))
            watx8f = cpool.tile([128, HOPS, 8], F32)
            nc.vector.memset(watx8f[:], 0.0)
            for h in range(HOPS):
                nc.vector.tensor_copy(watx8f[:, h, h:h + 1], watxf[:])
            watx8 = cpool.tile([128, HOPS, 8], F32R)
            nc.vector.tensor_copy(watx8[:], watx8f[:])

            # replicate the three PReLU alphas to [128, 3] via K=1 matmul
            al_f = cpool.tile([1, 3], F32)
            nc.sync.dma_start(al_f[0:1, 0:1], a_jk[:])
            nc.sync.dma_start(al_f[0:1, 1:2], a_main[:])
            nc.sync.dma_start(al_f[0:1, 2:3], a_out[:])
            ones_sb = cpool.tile([1, 128], F32)
            nc.vector.memset(ones_sb[:], 1.0)
            half_col = cpool.tile([128, 1], F32)
            nc.vector.memset(half_col[:], 0.5)
            al_ps = mmps.tile([128, 3], F32, tag="mm")
            nc.tensor.matmul(al_ps[:], ones_sb[:], al_f[:],
                             start=True, stop=True)
            alpha = cpool.tile([128, 3], F32)
            nc.scalar.activation(alpha[:], al_ps[:], AF.Copy)

            def prelu_from_psum(dst, ps_ap, a_col):
                """dst = prelu(ps, a) in one ACT op (parametric relu table)."""
                nc.scalar.activation(dst[:], ps_ap, AF.Prelu, alpha=a_col)

            # ---------------- main loop: 3-stage software pipeline ----------------
            # P1: load + transpose; P2: JK MLP + scores + softmax weights;
            # P3: aggregation + output FFN + store.  Emission is skewed so
            # each engine's in-order stream interleaves ~3 tiles.

            def phase1(st):
                n0, TT = st["n0"], st["TT"]
                B = TT // 128
                x_sb, xt_sb = [], []
                for h in range(HOPS):
                    x_h = xpool.tile([128, B, 128], XDT, tag="x")
                    nc.sync.dma_start(
                        x_h[:],
                        feats.ap()[h, n0:n0 + TT, :].rearrange(
                            "(b p) f -> p b f", p=128))
                    x_sb.append(x_h)
                    ps = xtps.tile([128, TT], XDT, tag="xtps")
                    for b in range(B):
                        nc.tensor.transpose(ps[:, b * 128:(b + 1) * 128],
                                            x_h[:, b, :],
                                            id_sb[:].bitcast(XDT))
                    xt_h = xtpool.tile([128, TT], F32R, tag="xt")
                    if h % 2 == 0:
                        nc.scalar.activation(xt_h[:], ps[:], AF.Copy)
                    else:
                        nc.vector.tensor_copy(xt_h[:], ps[:])
                    xt_sb.append(xt_h)
                st["x"], st["xt"] = x_sb, xt_sb

            def phase2(st):
                TT = st["TT"]
                B = TT // 128
                x_sb, xt_sb = st["x"], st["xt"]

                h1_sb = []
                for m in range(2):
                    ps = mmps.tile([128, TT], F32, tag="mm")
                    for h in range(HOPS):
                        nc.tensor.matmul(ps[:], w1r[:, h, m, :], xt_sb[h][:],
                                         start=(h == 0), stop=(h == HOPS - 1))
                    h1 = actpool.tile([128, TT], F32R, tag="h1")
                    prelu_from_psum(h1, ps[:], alpha[:, 0:1])
                    h1_sb.append(h1)

                jk_sb = []
                for m in range(2):
                    ps = mmps.tile([128, TT], F32, tag="mm")
                    for k in range(2):
                        nc.tensor.matmul(ps[:], w2r[:, k, m, :], h1_sb[k][:],
                                         start=(k == 0), stop=(k == 1))
                    jk = actpool.tile([128, TT], F32R, tag="jk")
                    prelu_from_psum(jk, ps[:], alpha[:, 1:2])
                    jk_sb.append(jk)

                s_ps = mmps.tile([8, TT], F32, tag="mm")
                for k in range(2):
                    nc.tensor.matmul(s_ps[:], wref8[:, k, :], jk_sb[k][:],
                                     start=(k == 0), stop=False,
                                     skip_group_check=True)
                for h in range(HOPS):
                    nc.tensor.matmul(s_ps[:], watx8[:, h, :], xt_sb[h][:],
                                     start=False, stop=(h == HOPS - 1),
                                     skip_group_check=True)
                # sigmoid(s) = 0.5 + 0.5*tanh(s/2); tanh shares the exp
                # activation-table set, so no table reloads per tile
                sg = smpool.tile([8, TT], F32, tag="sg")
                nc.scalar.activation(sg[:], s_ps[:], AF.Tanh, scale=0.5)

                e_ps = eps.tile([128, B * 8], F32, tag="eps")
                for b in range(B):
                    nc.tensor.transpose(e_ps[:, b * 8:(b + 1) * 8],
                                        sg[:, b * 128:(b + 1) * 128],
                                        id_sb[0:8, 0:8])
                e_sb = smpool.tile([128, B, 8], F32, tag="e")
                esum = smpool.tile([128, B], F32, tag="esum")
                r_sb = smpool.tile([128, B], F32, tag="r")
                ew = smpool.tile([128, B, 8], F32, tag="ew")
                for b in range(B):
                    nc.scalar.activation(e_sb[:, b, :],
                                         e_ps[:, b * 8:(b + 1) * 8],
                                         AF.Exp, scale=0.5, bias=half_col[:],
                                         accum_out=esum[:, b:b + 1])
                    nc.vector.reciprocal(r_sb[:, b:b + 1], esum[:, b:b + 1])
                    nc.vector.tensor_scalar(ew[:, b, :], e_sb[:, b, :],
                                            r_sb[:, b:b + 1], None,
                                            op0=ALU.mult)
                st["ew"] = ew

            def phase3(st):
                n0, TT = st["n0"], st["TT"]
                B = TT // 128
                x_sb, ew = st["x"], st["ew"]

                agg_blocks = []
                for b in range(B):
                    cur = aggpool.tile([128, 128], F32, tag="aggp")
                    nc.vector.tensor_scalar(cur[:],
                                            x_sb[0][:, b, :].bitcast(F32),
                                            ew[:, b, 0:1], None, op0=ALU.mult)
                    for h in range(1, HOPS):
                        nxt = aggpool.tile([128, 128],
                                           F32R if h == HOPS - 1 else F32,
                                           tag="aggp")
                        nc.vector.scalar_tensor_tensor(
                            nxt[:], x_sb[h][:, b, :].bitcast(F32),
                            ew[:, b, h:h + 1], cur[:],
                            op0=ALU.mult, op1=ALU.add)
                        cur = nxt
                    agg_blocks.append(cur)

                a_ps = mm3ps.tile([128, TT], F32R, tag="mm3")
                for b in range(B):
                    nc.tensor.transpose(a_ps[:, b * 128:(b + 1) * 128],
                                        agg_blocks[b][:],
                                        id_sb[:].bitcast(F32R))
                aggt = actpool.tile([128, TT], F32R, tag="aggt")
                nc.scalar.activation(aggt[:], a_ps[:], AF.Copy)

                o1_sb = []
                for m in range(2):
                    ps = mm3ps.tile([128, TT], F32, tag="mm3")
                    nc.tensor.matmul(ps[:], wo1r[:, m, :], aggt[:],
                                     start=True, stop=True)
                    o1 = actpool.tile([128, TT], F32R, tag="o1")
                    prelu_from_psum(o1, ps[:], alpha[:, 2:3])
                    o1_sb.append(o1)

                o_ps = mm3ps.tile([128, B * NCLS], F32, tag="mm3")
                for b in range(B):
                    for k in range(2):
                        nc.tensor.matmul(
                            o_ps[:, b * NCLS:(b + 1) * NCLS],
                            o1_sb[k][:, b * 128:(b + 1) * 128],
                            wo2r[:, k, :],
                            start=(k == 0), stop=(k == 1),
                            skip_group_check=True)
                out_sb = outpool.tile([128, B, NCLS], F32, tag="out")
                nc.scalar.activation(out_sb[:], o_ps[:], AF.Copy)
                nc.sync.dma_start(
                    out.ap()[n0:n0 + TT, :].rearrange("(b p) c -> p b c",
                                                      p=128),
                    out_sb[:])

            import contextlib
            loop_cm = tc.For_i(0, loop_k) if loop_k else contextlib.nullcontext()
            NT = len(TILES)
            states = {}
            with loop_cm:
                _run_tiles(phase1, phase2, phase3, states, NT)

    nc.compile()
    return nc


def _run_tiles(phase1, phase2, phase3, states, NT):
    if True:
        if True:
            for t in range(NT + 2):
                if t < NT:
                    n0, TT = TILES[t]
                    states[t] = {"n0": n0, "TT": TT}
                    phase1(states[t])
                if t >= 1 and t - 1 < NT:
                    phase2(states[t - 1])
                if t >= 2 and t - 2 < NT:
                    phase3(states[t - 2])
                    del states[t - 2]


def _get_program():
    if "nc" not in _CACHE:
        _CACHE["nc"] = _build_program()
    return _CACHE["nc"]


def kernel(**inputs):
    nc = _get_program()

    feats = np.asarray(inputs["feats"], dtype=np.float32)
    pad = NCORES * NPC - feats.shape[1]
    feats_p = np.pad(feats, ((0, 0), (0, pad), (0, 0)))

    def scal(name):
        return np.asarray(inputs[name], dtype=np.float32).reshape(1, 1)

    shared = {
        "W_jk1": np.ascontiguousarray(inputs["W_jk1"], dtype=np.float32),
        "W_jk2": np.ascontiguousarray(inputs["W_jk2"], dtype=np.float32),
        "w_att_ref": np.ascontiguousarray(inputs["w_att_ref"], dtype=np.float32),
        "w_att_x": np.ascontiguousarray(inputs["w_att_x"], dtype=np.float32),
        "W_o1": np.ascontiguousarray(inputs["W_o1"], dtype=np.float32),
        "W_o2": np.ascontiguousarray(inputs["W_o2"], dtype=np.float32),
        "a_jk": scal("a_jk"), "a_main": scal("a_main"), "a_out": scal("a_out"),
    }
    in_maps = []
    for c in range(NCORES):
        m = dict(shared)
        m["feats"] = np.ascontiguousarray(feats_p[:, c * NPC:(c + 1) * NPC, :])
        in_maps.append(m)

    res = run_bass_kernel_spmd(nc, in_maps, core_ids=list(range(NCORES)))
    out = np.concatenate([res.results[c]["out"] for c in range(NCORES)],
                         axis=0)[:N]
    return np.ascontiguousarray(out, dtype=np.float32)
